# revision 1
# baseline (speedup 1.0000x reference)
"""Trainium2 Bass kernel for BoundaryFocalLoss.

Full-input contract: kernel(**inputs) takes the complete arrays
(inputs [128,200000] f32, targets [128,200000] i32, mask [128,200000] f32)
and returns the scalar loss, distributing work over 8 NeuronCores by
sharding the T dimension (each core: all 128 batch rows x 25000 columns,
targets carry a 4/3-column halo for the 7-wide boundary window).

Math (exactly equivalent to the reference, validated in fp32 to ~1e-6):
    az  = |x|
    e   = exp(-az)
    L   = ln(1+e)            # = softplus(-|x|)
    R   = exp(-L)            # = sigmoid(|x|)
    bce = relu(x) - x*s + L  # s = 0.025 + 0.95*t
    pt  = exp(-bce)
    F   = (0.75-0.5t) * (1-pt)^2 * bce * (1+4*dilate7(trans)) * (1.5-R)
    loss = sum(F * mask) / sum(mask)

The per-element product F*? is reduced with the TensorEngine: the last two
factors (Vo = weights-product, bce) are contracted chunkwise via
matmul(lhsT=Vo, rhs=bce) accumulated in one PSUM bank; the diagonal of the
accumulated [CH,CH] matrix holds the per-column sums, extracted once at the
end with tensor_tensor_reduce against an identity matrix.

Most intermediates are bf16 (validated end-to-end rel err ~5e-5 vs the f32
reference on the real input distribution); the label-smoothing constants
enter as fp32 immediates inside scalar_tensor_tensor ops so no systematic
bias is introduced.
"""

import os
import numpy as np
from contextlib import ExitStack

P = 128          # partitions == batch rows
N_CORES = 8
HALO_L, HALO_R = 4, 3
HALO = HALO_L + HALO_R


def _build_program(T_shard, N, with_mask, CH=125, repeat=1):
    """Build + compile the single-core Bass program (SPMD across cores).

    repeat>1 wraps the whole tile loop in a device-side For_i so the body
    executes `repeat` times per launch — used only for wall-clock timing
    (the per-iteration delta isolates device time from host transfers).
    """
    from contextlib import nullcontext
    import concourse.bacc as bacc
    import concourse.tile as tile
    import concourse.mybir as mybir

    dt = mybir.dt
    Alu = mybir.AluOpType
    Act = mybir.ActivationFunctionType

    NT = T_shard // N
    assert NT * N == T_shard
    assert N % CH == 0 and N % 2 == 0
    n_chunks = N // CH

    # The stock act-table-load pass assigns each activation the FIRST
    # act_info set containing its function (Exp -> exp_and_others, Ln ->
    # natural_log), which thrashes ~2.7us table reloads every tile. All
    # functions used here (Exp/Ln/Square + fillers) co-reside in
    # natural_log_exp_and_others, so strip them from every other set's
    # advertised contents; the pass then lands everything on that one set
    # and a single load suffices. Indices (act_func_set_id) are unchanged,
    # so the emitted BIR still references the real act_info.json entry.
    import concourse.hw_specs as hw_specs
    import bass_rust as _bass_rust
    from concourse._compat import spectator_function

    _ONE_SET = "natural_log_exp_and_others"
    _USED = {
        mybir.ActivationFunctionType.Exp,
        mybir.ActivationFunctionType.Ln,
        mybir.ActivationFunctionType.Square,
        mybir.ActivationFunctionType.Copy,
        mybir.ActivationFunctionType.Identity,
    }

    class _OneActSetBacc(bacc.Bacc):
        def insert_act_table_loads(self):
            has_activation = any(
                isinstance(i, mybir.InstActivation)
                for b in self.main_func.blocks
                for i in b.instructions
            )
            if not has_activation:
                return
            tables = [
                (name, (funcs if name == _ONE_SET else funcs - _USED))
                for name, funcs in hw_specs.get_activation_tables(self.m.arch).items()
            ]
            _bass_rust.insert_act_table_loads(self, tables)

    nc = _OneActSetBacc("TRN2", target_bir_lowering=False, debug=False)

    x_d = nc.dram_tensor("x", [P, T_shard], dt.float32, kind="ExternalInput").ap()
    t_d = nc.dram_tensor("t", [P, T_shard + HALO], dt.int32, kind="ExternalInput").ap()
    eye_d = nc.dram_tensor("eye", [P, P], dt.float32, kind="ExternalInput").ap()
    if with_mask:
        m_d = nc.dram_tensor("m", [P, T_shard], dt.float32, kind="ExternalInput").ap()
    out_d = nc.dram_tensor("out", [P, 2], dt.float32, kind="ExternalOutput").ap()

    with tile.TileContext(nc) as tc, ExitStack() as ctx:
        io = ctx.enter_context(tc.tile_pool(name="io", bufs=3))
        val = ctx.enter_context(tc.tile_pool(name="val", bufs=2))
        val3 = ctx.enter_context(tc.tile_pool(name="val3", bufs=3))
        singles = ctx.enter_context(tc.tile_pool(name="singles", bufs=1))
        psum = ctx.enter_context(tc.tile_pool(name="psum", bufs=1, space="PSUM"))

        eye_sb = singles.tile([P, P], dt.float32)
        nc.sync.dma_start(eye_sb[:], eye_d[:])

        out_sb = singles.tile([P, 2], dt.float32)
        nc.vector.memset(out_sb[:], 0.0)

        if with_mask:
            ms = singles.tile([P, NT], dt.float32)

        acc = psum.tile([P, CH], dt.float32)

        rep_cm = tc.For_i(0, repeat, 1) if repeat > 1 else nullcontext()
        with rep_cm:
          for i in range(NT):
            c0 = i * N
            # ---- loads -------------------------------------------------
            x_t = io.tile([P, N], dt.float32, tag="x")
            nc.sync.dma_start(x_t[:], x_d[:, c0:c0 + N])
            t_t = io.tile([P, N + HALO], dt.int32, tag="t")
            nc.sync.dma_start(t_t[:], t_d[:, c0:c0 + N + HALO])
            if with_mask:
                m_t = io.tile([P, N], dt.float32, tag="m")
                nc.sync.dma_start(m_t[:], m_d[:, c0:c0 + N])

            # ---- dtype staging (GpSimd measured ~4us/copy: avoid Pool) ----
            Xb = val3.tile([P, N], dt.bfloat16, tag="Xb")
            nc.scalar.activation(Xb[:], x_t[:], Act.Copy)
            tb = val3.tile([P, N + HALO], dt.bfloat16, tag="tb")
            nc.vector.tensor_copy(tb[:], t_t[:])

            # ---- transcendental chain (one ACT table set) -------------
            az = val3.tile([P, N], dt.bfloat16, tag="az")           # DVE |x| via bit-and on packed pairs
            nc.vector.tensor_scalar(
                az[:].bitcast(dt.uint32), Xb[:].bitcast(dt.uint32),
                0x7FFF7FFF, None, Alu.bitwise_and)
            e = val3.tile([P, N], dt.bfloat16, tag="e")
            nc.scalar.activation(e[:], az[:], Act.Exp, scale=-1.0)
            L = val3.tile([P, N], dt.bfloat16, tag="L")
            nc.scalar.activation(L[:], e[:], Act.Ln, bias=1.0)
            R = val.tile([P, N], dt.bfloat16, tag="R")
            nc.scalar.activation(R[:], L[:], Act.Exp, scale=-1.0)

            # ---- bce = relu(x) - 0.025x - 0.95*x*t + L ----------------
            tb_c = tb[:, HALO_L:HALO_L + N]
            xtf95 = val.tile([P, N], dt.bfloat16, tag="xtf95")     # -0.95*x*t
            nc.vector.scalar_tensor_tensor(
                xtf95[:], Xb[:], -0.95, tb_c, Alu.mult, Alu.mult)
            rXbL = val.tile([P, N], dt.bfloat16, tag="rXbL")       # relu(x) + L
            nc.vector.scalar_tensor_tensor(
                rXbL[:], Xb[:], 0.0, L[:], Alu.max, Alu.add)
            vL = val.tile([P, N], dt.bfloat16, tag="vL")           # -0.025x + relu(x) + L
            nc.vector.scalar_tensor_tensor(
                vL[:], Xb[:], -0.025, rXbL[:], Alu.mult, Alu.add)
            bce = val.tile([P, N], dt.bfloat16, tag="bce")
            nc.vector.tensor_tensor(bce[:], xtf95[:], vL[:], Alu.add)

            pt = val.tile([P, N], dt.float32, tag="pt")
            nc.scalar.activation(pt[:], bce[:], Act.Exp, scale=-1.0)
            omp2 = val.tile([P, N], dt.bfloat16, tag="omp2")       # (1-pt)^2
            nc.scalar.activation(omp2[:], pt[:], Act.Square, bias=1.0, scale=-1.0)

            # ---- boundary dilation (7-wide window of transitions) -----
            # TR[j] = (t[j+1] != t[j]); output col c needs max(TR[c..c+6]).
            # Log-doubling: d1 covers 2, d2 covers 4, d3 covers 7.
            TR = val.tile([P, N + 6], dt.bfloat16, tag="TR")
            nc.vector.tensor_tensor(
                TR[:], tb[:, 1:N + 7], tb[:, 0:N + 6], Alu.not_equal)
            d1 = val.tile([P, N + 5], dt.bfloat16, tag="d1")
            nc.vector.tensor_tensor(
                d1[:], TR[:, 0:N + 5], TR[:, 1:N + 6], Alu.max)
            d2 = val.tile([P, N + 3], dt.bfloat16, tag="d2")
            nc.vector.tensor_tensor(
                d2[:], d1[:, 0:N + 3], d1[:, 2:N + 5], Alu.max)
            d3 = val.tile([P, N], dt.bfloat16, tag="d3")
            nc.vector.tensor_tensor(
                d3[:], d2[:, 0:N], d2[:, 3:N + 3], Alu.max)
            W = val.tile([P, N], dt.bfloat16, tag="W")             # 1 + 4*d3
            nc.vector.tensor_scalar(W[:], d3[:], 4.0, 1.0, Alu.mult, Alu.add)

            # ---- focal weights ----------------------------------------
            ada = val.tile([P, N], dt.bfloat16, tag="ada")         # 1.5 - R
            nc.vector.tensor_scalar(ada[:], R[:], -1.0, 1.5, Alu.mult, Alu.add)
            aw = val.tile([P, N], dt.bfloat16, tag="aw")           # 0.75 - 0.5t
            nc.vector.tensor_scalar(aw[:], tb_c, -0.5, 0.75, Alu.mult, Alu.add)
            V1 = val.tile([P, N], dt.bfloat16, tag="V1")
            nc.vector.tensor_tensor(V1[:], aw[:], ada[:], Alu.mult)
            V = val.tile([P, N], dt.bfloat16, tag="V")
            nc.vector.tensor_tensor(V[:], V1[:], W[:], Alu.mult)
            Vo = val.tile([P, N], dt.bfloat16, tag="Vo")
            nc.vector.tensor_tensor(Vo[:], V[:], omp2[:], Alu.mult)

            rhs = bce
            if with_mask:
                bm = val.tile([P, N], dt.bfloat16, tag="bm")
                nc.vector.tensor_tensor(bm[:], bce[:], m_t[:], Alu.mult)
                rhs = bm
                nc.vector.tensor_reduce(
                    ms[:, i:i + 1], m_t[:], axis=mybir.AxisListType.X, op=Alu.add)

            # ---- PE contraction: acc[m,n] += sum_b Vo[b,m]*rhs[b,n] ----
            for c in range(n_chunks):
                s0 = c * CH
                nc.tensor.matmul(
                    acc[0:CH, 0:CH],
                    Vo[:, s0:s0 + CH],
                    rhs[:, s0:s0 + CH],
                    start=(i == 0 and c == 0),
                    stop=(i == NT - 1 and c == n_chunks - 1),
                )

        # ---- tail: diagonal of acc holds per-column sums --------------
        accsb = singles.tile([P, CH], dt.float32)
        nc.vector.tensor_copy(accsb[0:CH, :], acc[0:CH, 0:CH])
        diag = singles.tile([P, CH], dt.float32)
        nc.vector.tensor_tensor(
            diag[0:CH, :], accsb[0:CH, :], eye_sb[0:CH, 0:CH], Alu.mult)
        nc.vector.tensor_reduce(
            out_sb[0:CH, 0:1], diag[0:CH, :], axis=mybir.AxisListType.X, op=Alu.add)
        if with_mask:
            nc.vector.tensor_reduce(
                out_sb[:, 1:2], ms[:], axis=mybir.AxisListType.X, op=Alu.add)
        nc.sync.dma_start(out_d[:], out_sb[:])

    nc.compile()
    return nc


def _build_program_v2(T_shard, N, CH=125, repeat=1):
    """N=2500 no-mask variant: half the instructions of v1 (per-instr
    fixed costs dominate at N=1250), SBUF fitted via lifetime-based tag
    sharing. Targets int32 are consumed directly (no bf16 staging)."""
    from contextlib import nullcontext
    import concourse.bacc as bacc
    import concourse.tile as tile
    import concourse.mybir as mybir

    dt = mybir.dt
    Alu = mybir.AluOpType
    Act = mybir.ActivationFunctionType

    NT = T_shard // N
    assert NT * N == T_shard
    assert N % CH == 0 and N % 2 == 0
    n_chunks = N // CH

    import concourse.hw_specs as hw_specs
    import bass_rust as _bass_rust

    _ONE_SET = "natural_log_exp_and_others"
    _USED = {
        mybir.ActivationFunctionType.Exp,
        mybir.ActivationFunctionType.Ln,
        mybir.ActivationFunctionType.Square,
        mybir.ActivationFunctionType.Copy,
        mybir.ActivationFunctionType.Identity,
    }

    class _OneActSetBacc(bacc.Bacc):
        def insert_act_table_loads(self):
            has_activation = any(
                isinstance(i, mybir.InstActivation)
                for b in self.main_func.blocks
                for i in b.instructions
            )
            if not has_activation:
                return
            tables = [
                (name, (funcs if name == _ONE_SET else funcs - _USED))
                for name, funcs in hw_specs.get_activation_tables(self.m.arch).items()
            ]
            _bass_rust.insert_act_table_loads(self, tables)

    nc = _OneActSetBacc("TRN2", target_bir_lowering=False, debug=False)

    x_d = nc.dram_tensor("x", [P, T_shard], dt.float32, kind="ExternalInput").ap()
    t_d = nc.dram_tensor("t", [P, T_shard + HALO], dt.int32, kind="ExternalInput").ap()
    eye_d = nc.dram_tensor("eye", [P, P], dt.float32, kind="ExternalInput").ap()
    out_d = nc.dram_tensor("out", [P, 2], dt.float32, kind="ExternalOutput").ap()

    with tile.TileContext(nc) as tc, ExitStack() as ctx:
        io = ctx.enter_context(tc.tile_pool(name="io", bufs=2))
        val = ctx.enter_context(tc.tile_pool(name="val", bufs=2))
        singles = ctx.enter_context(tc.tile_pool(name="singles", bufs=1))
        psum = ctx.enter_context(tc.tile_pool(name="psum", bufs=1, space="PSUM"))

        eye_sb = singles.tile([P, P], dt.float32)
        nc.sync.dma_start(eye_sb[:], eye_d[:])
        out_sb = singles.tile([P, 2], dt.float32)
        nc.vector.memset(out_sb[:], 0.0)
        acc = psum.tile([P, CH], dt.float32)

        rep_cm = tc.For_i(0, repeat, 1) if repeat > 1 else nullcontext()
        with rep_cm:
          for i in range(NT):
            c0 = i * N
            x_t = io.tile([P, N], dt.float32, tag="x")
            nc.sync.dma_start(x_t[:], x_d[:, c0:c0 + N])
            t_t = io.tile([P, N + HALO], dt.int32, tag="t")
            nc.sync.dma_start(t_t[:], t_d[:, c0:c0 + N + HALO])
            t_c = t_t[:, HALO_L:HALO_L + N]

            Xb = val.tile([P, N], dt.bfloat16, tag="Xb")
            nc.scalar.activation(Xb[:], x_t[:], Act.Copy)

            az = val.tile([P, N], dt.bfloat16, tag="tmp1")   # shares with d2
            nc.vector.tensor_scalar(
                az[:].bitcast(dt.uint32), Xb[:].bitcast(dt.uint32),
                0x7FFF7FFF, None, Alu.bitwise_and)
            e = val.tile([P, N], dt.bfloat16, tag="tmp2")    # shares with d3
            nc.scalar.activation(e[:], az[:], Act.Exp, scale=-1.0)
            L = val.tile([P, N], dt.bfloat16, tag="L")
            nc.scalar.activation(L[:], e[:], Act.Ln, bias=1.0)
            R = val.tile([P, N], dt.bfloat16, tag="R")
            nc.scalar.activation(R[:], L[:], Act.Exp, scale=-1.0)

            xtf95 = val.tile([P, N], dt.bfloat16, tag="tmp3")  # shares with W
            nc.vector.scalar_tensor_tensor(
                xtf95[:], Xb[:], -0.95, t_c, Alu.mult, Alu.mult)
            rXbL = val.tile([P, N], dt.bfloat16, tag="tmp4")   # shares with ada
            nc.vector.scalar_tensor_tensor(
                rXbL[:], Xb[:], 0.0, L[:], Alu.max, Alu.add)
            vL = val.tile([P, N], dt.bfloat16, tag="tmp5")     # shares with aw
            nc.vector.scalar_tensor_tensor(
                vL[:], Xb[:], -0.025, rXbL[:], Alu.mult, Alu.add)
            bce = val.tile([P, N], dt.bfloat16, tag="bce")
            nc.vector.tensor_tensor(bce[:], xtf95[:], vL[:], Alu.add)

            pt = val.tile([P, N], dt.float32, tag="pt")
            nc.scalar.activation(pt[:], bce[:], Act.Exp, scale=-1.0)
            omp2 = val.tile([P, N], dt.bfloat16, tag="omp2")
            nc.scalar.activation(omp2[:], pt[:], Act.Square, bias=1.0, scale=-1.0)

            TR = val.tile([P, N + 6], dt.bfloat16, tag="tmp6")  # shares with V1
            nc.vector.tensor_tensor(
                TR[:], t_t[:, 1:N + 7], t_t[:, 0:N + 6], Alu.not_equal)
            d1 = val.tile([P, N + 5], dt.bfloat16, tag="tmp7")  # shares with V
            nc.vector.tensor_tensor(
                d1[:], TR[:, 0:N + 5], TR[:, 1:N + 6], Alu.max)
            d2 = val.tile([P, N + 3], dt.bfloat16, tag="tmp1")
            nc.vector.tensor_tensor(
                d2[:], d1[:, 0:N + 3], d1[:, 2:N + 5], Alu.max)
            d3 = val.tile([P, N], dt.bfloat16, tag="tmp2")
            nc.vector.tensor_tensor(
                d3[:], d2[:, 0:N], d2[:, 3:N + 3], Alu.max)
            W = val.tile([P, N], dt.bfloat16, tag="tmp3")
            nc.vector.tensor_scalar(W[:], d3[:], 4.0, 1.0, Alu.mult, Alu.add)

            ada = val.tile([P, N], dt.bfloat16, tag="tmp4")
            nc.vector.tensor_scalar(ada[:], R[:], -1.0, 1.5, Alu.mult, Alu.add)
            aw = val.tile([P, N], dt.bfloat16, tag="tmp5")
            nc.vector.tensor_scalar(aw[:], t_c, -0.5, 0.75, Alu.mult, Alu.add)
            V1 = val.tile([P, N], dt.bfloat16, tag="tmp6")
            nc.vector.tensor_tensor(V1[:], aw[:], ada[:], Alu.mult)
            V = val.tile([P, N], dt.bfloat16, tag="tmp7")
            nc.vector.tensor_tensor(V[:], V1[:], W[:], Alu.mult)
            Vo = val.tile([P, N], dt.bfloat16, tag="Vo")
            nc.vector.tensor_tensor(Vo[:], V[:], omp2[:], Alu.mult)

            for c in range(n_chunks):
                s0 = c * CH
                nc.tensor.matmul(
                    acc[0:CH, 0:CH],
                    Vo[:, s0:s0 + CH],
                    bce[:, s0:s0 + CH],
                    start=(i == 0 and c == 0),
                    stop=(i == NT - 1 and c == n_chunks - 1),
                )

        accsb = singles.tile([P, CH], dt.float32)
        nc.vector.tensor_copy(accsb[0:CH, :], acc[0:CH, 0:CH])
        diag = singles.tile([P, CH], dt.float32)
        nc.vector.tensor_tensor(
            diag[0:CH, :], accsb[0:CH, :], eye_sb[0:CH, 0:CH], Alu.mult)
        nc.vector.tensor_reduce(
            out_sb[0:CH, 0:1], diag[0:CH, :], axis=mybir.AxisListType.X, op=Alu.add)
        nc.sync.dma_start(out_d[:], out_sb[:])

    nc.compile()
    return nc


_PROGRAM_CACHE = {}


def _get_program(T_shard, N, with_mask):
    key = (T_shard, N, with_mask)
    if key not in _PROGRAM_CACHE:
        _PROGRAM_CACHE[key] = _build_program(T_shard, N, with_mask)
    return _PROGRAM_CACHE[key]


def kernel(inputs, targets, mask):
    from concourse.bass_utils import run_bass_kernel_spmd

    x = np.ascontiguousarray(np.asarray(inputs, dtype=np.float32))
    t = np.ascontiguousarray(np.asarray(targets, dtype=np.int32))
    m = np.ascontiguousarray(np.asarray(mask, dtype=np.float32))
    Bq, T = x.shape
    assert Bq == P and T % N_CORES == 0
    T_shard = T // N_CORES
    N = 1250
    ones_mask = bool(m.min() == 1.0 and m.max() == 1.0)

    nc = _get_program(T_shard, N, with_mask=not ones_mask)

    t_pad = np.pad(t, ((0, 0), (HALO_L, HALO_R)), mode="edge")
    eye = np.eye(P, dtype=np.float32)
    in_maps = []
    for c in range(N_CORES):
        lo = c * T_shard
        im = {
            "x": np.ascontiguousarray(x[:, lo:lo + T_shard]),
            "t": np.ascontiguousarray(t_pad[:, lo:lo + T_shard + HALO]),
            "eye": eye,
        }
        if not ones_mask:
            im["m"] = np.ascontiguousarray(m[:, lo:lo + T_shard])
        in_maps.append(im)

    res = run_bass_kernel_spmd(nc, in_maps, core_ids=list(range(N_CORES)))
    outs = [r["out"] for r in res.results]

    loss = float(sum(o[:, 0].astype(np.float64).sum() for o in outs))
    if ones_mask:
        msum = float(Bq) * float(T)
    else:
        msum = float(sum(o[:, 1].astype(np.float64).sum() for o in outs))
    if msum <= 0.0:
        return np.float32(0.0)
    return np.float32(loss / msum)

